# revision 14
# baseline (speedup 1.0000x reference)
"""LocalBandSimilarityBlock — Trainium2 Bass kernel, 8-way sequence-parallel.

N=6144 nodes, D=512. Each of the 8 cores owns R=N/8=768 query rows and
streams all keys/values (grid positions are random, so the radius band is
unstructured — K/V are fully replicated per the sharding hint).

Per-core pipeline (all compute on device, host only shards/casts/concats):
  P1  LN(x) -> h (bf16) streamed per 128-row tile; PE-transpose to hT;
      row norms -> rn written to a DRAM row for later free-dim broadcast.
  P2  q-side: LN(xq) -> hq, hqT; qAT = [ (hq@Wq+bq)*scale ; hn_q ]^T.
  P3  B-side: kT = Wk^T@hT + bk, hnT = hT*rn; packed to DRAM as BT
      [kt][p][dt][j] for streaming; v = h@Wv + bv -> DRAM per k-tile.
  P4  attention, q in 2 halves (PSUM budget): per k-tile stream BT/v,
      logitsT psum = BT^T @ qAT (contract 1024), band mask via Chebyshev
      distance on DVE, p = exp(logits)*mask (no max-subtraction needed:
      logits are O(5) bounded), out/den accumulate via PE over 48 k-tiles.
  P5  o@Wo + residual -> x2; LN2 -> h2T.
  P6  FFN: aT = gelu(W1^T@h2 + b1) built transposed (no transpose pass),
      y = aT^T@W2 + b2 + x2 -> out.
"""

import numpy as np

N = 6144
D = 512
R = N // 8          # 768 query rows per core
NT = N // 128       # 48 k-tiles
RT = R // 128       # 6 q-tiles per core
RH = R // 2         # q-half width (PSUM budget)
JH = RH // 128      # q-tiles per half
NB = N // 512       # 12 node blocks
DT = D // 128       # 4
FT = (4 * D) // 128 # 16 (FFN hidden tiles)
RADIUS = 2.0
SIM_BETA = 1.0
SCALE = 1.0 / np.sqrt(np.float32(D))
LN_EPS = 1e-5
COS_EPS = 1e-8

_PROG_CACHE = {}


def _build_program(ln1_identity: bool, ln2_identity: bool, gelu_mode: str = "hw"):
    import concourse.bass as bass
    import concourse.bacc as bacc
    import concourse.tile as tile
    import concourse.mybir as mybir
    from concourse.masks import make_identity

    fp32 = mybir.dt.float32
    bf16 = mybir.dt.bfloat16
    AF = mybir.ActivationFunctionType
    OP = mybir.AluOpType
    AX = mybir.AxisListType

    nc = bacc.Bacc("TRN2", target_bir_lowering=False, debug=False, num_devices=8)

    # ---------------- external I/O ----------------
    x_bf = nc.dram_tensor("x_bf", [N, D], bf16, kind="ExternalInput")
    xq = nc.dram_tensor("xq", [R, D], fp32, kind="ExternalInput")
    gqx = nc.dram_tensor("gqx", [1, R], bf16, kind="ExternalInput")
    gqy = nc.dram_tensor("gqy", [1, R], bf16, kind="ExternalInput")
    qidx = nc.dram_tensor("qidx", [1, R], fp32, kind="ExternalInput")
    gkx = nc.dram_tensor("gkx", [N, 1], fp32, kind="ExternalInput")
    gky = nc.dram_tensor("gky", [N, 1], fp32, kind="ExternalInput")
    kidx = nc.dram_tensor("kidx", [N, 1], fp32, kind="ExternalInput")
    Wq = nc.dram_tensor("Wq", [D, D], bf16, kind="ExternalInput")
    Wk = nc.dram_tensor("Wk", [D, D], bf16, kind="ExternalInput")
    Wv = nc.dram_tensor("Wv", [D, D], bf16, kind="ExternalInput")
    Wo = nc.dram_tensor("Wo", [D, D], bf16, kind="ExternalInput")
    W1 = nc.dram_tensor("W1", [D, 4 * D], bf16, kind="ExternalInput")
    W2 = nc.dram_tensor("W2", [4 * D, D], bf16, kind="ExternalInput")
    bq_c = nc.dram_tensor("bq_c", [D, 1], fp32, kind="ExternalInput")
    bk_c = nc.dram_tensor("bk_c", [D, 1], fp32, kind="ExternalInput")
    b1_c = nc.dram_tensor("b1_c", [4 * D, 1], fp32, kind="ExternalInput")
    bv_r = nc.dram_tensor("bv_r", [1, D], bf16, kind="ExternalInput")
    bo_r = nc.dram_tensor("bo_r", [1, D], bf16, kind="ExternalInput")
    b2_r = nc.dram_tensor("b2_r", [1, D], bf16, kind="ExternalInput")
    if not ln1_identity:
        g1_r = nc.dram_tensor("g1_r", [1, D], fp32, kind="ExternalInput")
        bb1_r = nc.dram_tensor("bb1_r", [1, D], fp32, kind="ExternalInput")
    if not ln2_identity:
        g2_r = nc.dram_tensor("g2_r", [1, D], fp32, kind="ExternalInput")
        bb2_r = nc.dram_tensor("bb2_r", [1, D], fp32, kind="ExternalInput")
    out = nc.dram_tensor("out", [R, D], fp32, kind="ExternalOutput")

    with tile.TileContext(nc) as tc:
        with (
            tc.tile_pool(name="const", bufs=1) as constp,
            tc.tile_pool(name="stat", bufs=1) as statp,
            tc.tile_pool(name="wts", bufs=8) as wpool,
            tc.tile_pool(name="stream", bufs=3) as streamp,
            tc.tile_pool(name="work", bufs=3) as workp,
            tc.tile_pool(name="mini", bufs=6) as minip,
            tc.tile_pool(name="pacc", bufs=4, space="PSUM") as pacc,
            tc.tile_pool(name="pwork", bufs=2, space="PSUM") as pwork,
            tc.tile_pool(name="ptp", bufs=2, space="PSUM") as ptp,
            tc.tile_pool(name="dram", bufs=1, space="DRAM") as dramp,
        ):
            # ---------------- constants ----------------
            ident = constp.tile([128, 128], bf16, tag="ident")
            make_identity(nc, ident)
            ones_col = constp.tile([128, 1], bf16, tag="ones_col")
            nc.vector.memset(ones_col, 1.0)
            ones_row = constp.tile([1, 128], bf16, tag="ones_row")
            nc.vector.memset(ones_row, 1.0)
            eps_t = constp.tile([128, 1], fp32, tag="eps_t")
            nc.vector.memset(eps_t, float(LN_EPS))

            gqx_bc = constp.tile([128, R], bf16, tag="gqx_bc")
            nc.sync.dma_start(out=gqx_bc, in_=gqx[0:1, :].to_broadcast([128, R]))
            gqy_bc = constp.tile([128, R], bf16, tag="gqy_bc")
            nc.sync.dma_start(out=gqy_bc, in_=gqy[0:1, :].to_broadcast([128, R]))
            qidx_bc = constp.tile([128, R], fp32, tag="qidx_bc")
            nc.sync.dma_start(out=qidx_bc, in_=qidx[0:1, :].to_broadcast([128, R]))

            bqs3 = constp.tile([128, DT, 1], fp32, tag="bqs")
            nc.sync.dma_start(out=bqs3, in_=bq_c.rearrange("(a p) b -> p a b", p=128))
            bks3 = constp.tile([128, DT, 1], fp32, tag="bks")
            nc.sync.dma_start(out=bks3, in_=bk_c.rearrange("(a p) b -> p a b", p=128))
            bqs = bqs3.rearrange("p a b -> p (a b)")
            bks = bks3.rearrange("p a b -> p (a b)")
            # pre-scale bq by SCALE (q side of logits carries the 1/sqrt(D))
            nc.scalar.mul(bqs, bqs, float(SCALE))
            b1s3 = constp.tile([128, FT, 1], fp32, tag="b1s")
            nc.sync.dma_start(out=b1s3, in_=b1_c.rearrange("(a p) b -> p a b", p=128))
            b1s = b1s3.rearrange("p a b -> p (a b)")
            bvr = constp.tile([1, D], bf16, tag="bvr")
            nc.sync.dma_start(out=bvr, in_=bv_r[:, :])
            bor = constp.tile([1, D], bf16, tag="bor")
            nc.sync.dma_start(out=bor, in_=bo_r[:, :])
            b2r = constp.tile([1, D], bf16, tag="b2r")
            nc.sync.dma_start(out=b2r, in_=b2_r[:, :])
            if not ln1_identity:
                g1_bc = constp.tile([128, D], fp32, tag="g1_bc")
                nc.sync.dma_start(out=g1_bc, in_=g1_r[0:1, :].to_broadcast([128, D]))
                bb1_bc = constp.tile([128, D], fp32, tag="bb1_bc")
                nc.sync.dma_start(out=bb1_bc, in_=bb1_r[0:1, :].to_broadcast([128, D]))
            if not ln2_identity:
                g2_bc = constp.tile([128, D], fp32, tag="g2_bc")
                nc.sync.dma_start(out=g2_bc, in_=g2_r[0:1, :].to_broadcast([128, D]))
                bb2_bc = constp.tile([128, D], fp32, tag="bb2_bc")
                nc.sync.dma_start(out=bb2_bc, in_=bb2_r[0:1, :].to_broadcast([128, D]))

            # weights resident per phase via shared-slot pool
            def load_w(dram, n_tiles, width, tag):
                ts_ = []
                for i in range(n_tiles):
                    t = wpool.tile([128, 2048], bf16, tag="w", name=f"{tag}{i}")[:, :width]
                    nc.gpsimd.dma_start(out=t, in_=dram[i * 128 : (i + 1) * 128, :])
                    ts_.append(t)
                return ts_

            # DRAM scratch (tile-tracked for RAW deps)
            bt_dram = dramp.tile([NT, 128, 8, 128], bf16, tag="bt_dram")
            v_dram = dramp.tile([NT, 128, D], bf16, tag="v_dram")
            rn_row = dramp.tile([1, N], fp32, tag="rn_row")
            rnq_row = dramp.tile([1, R], fp32, tag="rnq_row")

            # LN helper: x_tile [128, D] (any dtype) -> h bf16 [128, D], plus
            # 1/max(||h||, eps). Returns (h_tile, rn_tile).
            def layer_norm(x_t, rows, identity_affine, g_bc, b_bc):
                stats = minip.tile([128, 6], fp32, tag="stats")
                mv = minip.tile([128, 2], fp32, tag="mv")
                nc.vector.bn_stats(out=stats[:rows], in_=x_t[:rows])
                nc.vector.bn_aggr(out=mv[:rows], in_=stats[:rows])
                rstd = minip.tile([128, 1], fp32, tag="rstd")
                nc.scalar.activation(out=rstd[:rows], in_=mv[:rows, 1:2], func=AF.Sqrt, bias=eps_t[:rows])
                nc.vector.reciprocal(out=rstd[:rows], in_=rstd[:rows])
                h_t = workp.tile([128, D], bf16, tag="h_t")
                nc.vector.tensor_scalar(
                    out=h_t[:rows], in0=x_t[:rows],
                    scalar1=mv[:rows, 0:1], scalar2=rstd[:rows],
                    op0=OP.subtract, op1=OP.mult,
                )
                if not identity_affine:
                    nc.vector.tensor_tensor(out=h_t[:rows], in0=h_t[:rows], in1=g_bc[:rows], op=OP.mult)
                    nc.vector.tensor_tensor(out=h_t[:rows], in0=h_t[:rows], in1=b_bc[:rows], op=OP.add)
                sq = workp.tile([128, D], fp32, tag="sq")
                nsq = minip.tile([128, 1], fp32, tag="nsq")
                nc.vector.tensor_tensor(out=sq[:rows], in0=h_t[:rows], in1=h_t[:rows], op=OP.mult)
                nc.vector.reduce_sum(out=nsq[:rows], in_=sq[:rows], axis=AX.X)
                rn = minip.tile([128, 1], fp32, tag="rn")
                nc.scalar.activation(out=rn[:rows], in_=nsq[:rows], func=AF.Sqrt)
                nc.vector.tensor_scalar_max(out=rn[:rows], in0=rn[:rows], scalar1=float(COS_EPS))
                nc.vector.reciprocal(out=rn[:rows], in_=rn[:rows])
                return h_t, rn

            # PE transpose h_t[:, dt*128:+128] -> dest[:, col:col+128]
            def transpose_128(src, dest_ap):
                ps = ptp.tile([128, 128], bf16, tag="tp")
                nc.tensor.transpose(ps, src, ident)
                nc.scalar.copy(out=dest_ap, in_=ps)

            # ---------------- P2: q side ----------------
            g1b = None if ln1_identity else g1_bc
            b1b = None if ln1_identity else bb1_bc
            hqT = statp.tile([128, DT, R], bf16, tag="hqT")
            rnq_t = statp.tile([128, RT], fp32, tag="rnq_t")
            for qt in range(RT):
                xq_t = workp.tile([128, D], fp32, tag="xq_t")
                nc.gpsimd.dma_start(out=xq_t, in_=xq[qt * 128 : (qt + 1) * 128, :])
                hq_t, rn_t = layer_norm(xq_t, 128, ln1_identity, g1b, b1b)
                nc.vector.tensor_copy(out=rnq_t[:, qt : qt + 1], in_=rn_t)
                for dt in range(DT):
                    transpose_128(hq_t[:, dt * 128 : (dt + 1) * 128], hqT[:, dt, qt * 128 : (qt + 1) * 128])
                nc.gpsimd.dma_start(out=rnq_row[0, qt * 128 : (qt + 1) * 128], in_=rn_t)

            wq_sb = load_w(Wq, DT, D, "wq")
            qAT = statp.tile([128, 8, R], bf16, tag="qAT")
            for dt in range(DT):
                for c0, cw in ((0, 512), (512, R - 512)) if R > 512 else ((0, R),):
                    ps = pwork.tile([128, 512], fp32, tag="pwork")
                    for din in range(DT):
                        nc.tensor.matmul(
                            ps[:, :cw], wq_sb[din][:, dt * 128 : (dt + 1) * 128],
                            hqT[:, din, c0 : c0 + cw],
                            start=(din == 0), stop=(din == DT - 1),
                        )
                    nc.vector.tensor_scalar(
                        out=qAT[:, dt, c0 : c0 + cw], in0=ps[:, :cw],
                        scalar1=float(SCALE), scalar2=bqs[:, dt : dt + 1],
                        op0=OP.mult, op1=OP.add,
                    )
            rnq_bc = constp.tile([128, R], fp32, tag="rnq_bc")
            nc.gpsimd.dma_start(out=rnq_bc, in_=rnq_row[0:1, :].to_broadcast([128, R]))
            if SIM_BETA != 1.0:
                nc.vector.tensor_scalar_mul(out=rnq_bc, in0=rnq_bc, scalar1=float(SIM_BETA))
            for dt in range(DT):
                nc.vector.tensor_tensor(out=qAT[:, DT + dt, :], in0=hqT[:, dt, :], in1=rnq_bc, op=OP.mult)

            # v_own (isolated-row fallback): v rows of this core = hq@Wv+bv
            wv_sb = load_w(Wv, DT, D, "wv")
            v_own = statp.tile([128, RT, D], bf16, tag="v_own")
            for qt in range(RT):
                ps = pwork.tile([128, 512], fp32, tag="pwork")
                for din in range(DT):
                    nc.tensor.matmul(
                        ps, hqT[:, din, qt * 128 : (qt + 1) * 128], wv_sb[din],
                        start=(din == 0), stop=False,
                    )
                nc.tensor.matmul(ps, ones_row, bvr, start=False, stop=True)
                nc.scalar.copy(out=v_own[:, qt, :], in_=ps)

            # ---------------- P1+P3: B side (h, hT, kT, hnT, v) ----------------
            wk_sb = load_w(Wk, DT, D, "wk")
            for nb in range(NB):
                hT_nb = workp.tile([128, DT, 512], bf16, tag="hT_nb", bufs=2)
                for j in range(4):
                    kt = nb * 4 + j
                    x_t = workp.tile([128, D], bf16, tag="x_t")
                    nc.gpsimd.dma_start(out=x_t, in_=x_bf[kt * 128 : (kt + 1) * 128, :])
                    h_t, rn_t = layer_norm(x_t, 128, ln1_identity, g1b, b1b)
                    nc.gpsimd.dma_start(out=rn_row[0, kt * 128 : (kt + 1) * 128], in_=rn_t)
                    for dt in range(DT):
                        transpose_128(h_t[:, dt * 128 : (dt + 1) * 128], hT_nb[:, dt, j * 128 : (j + 1) * 128])
                # kT -> BT[dt 0..3]
                for dt in range(DT):
                    ps = pwork.tile([128, 512], fp32, tag="pwork")
                    for din in range(DT):
                        nc.tensor.matmul(
                            ps, wk_sb[din][:, dt * 128 : (dt + 1) * 128], hT_nb[:, din, :],
                            start=(din == 0), stop=(din == DT - 1),
                        )
                    kT_sb = workp.tile([128, 4, 128], bf16, tag="kT_sb")
                    nc.scalar.activation(out=kT_sb.rearrange("p a b -> p (a b)"), in_=ps, func=AF.Identity, bias=bks[:, dt : dt + 1])
                    nc.gpsimd.dma_start(
                        out=bt_dram[nb * 4 : nb * 4 + 4, :, dt, :].rearrange("a p b -> p a b"),
                        in_=kT_sb,
                    )
                # hnT -> BT[dt 4..7]
                rn_bc = workp.tile([128, 512], fp32, tag="rn_bc")
                nc.gpsimd.dma_start(out=rn_bc, in_=rn_row[0:1, nb * 512 : (nb + 1) * 512].to_broadcast([128, 512]))
                for dt in range(DT):
                    hnT_sb = workp.tile([128, 4, 128], bf16, tag="hnT_sb")
                    nc.vector.tensor_tensor(
                        out=hnT_sb.rearrange("p a b -> p (a b)"), in0=hT_nb[:, dt, :], in1=rn_bc, op=OP.mult,
                    )
                    nc.gpsimd.dma_start(
                        out=bt_dram[nb * 4 : nb * 4 + 4, :, DT + dt, :].rearrange("a p b -> p a b"),
                        in_=hnT_sb,
                    )
                # v tiles
                for j in range(4):
                    kt = nb * 4 + j
                    ps = pwork.tile([128, 512], fp32, tag="pwork")
                    for din in range(DT):
                        nc.tensor.matmul(
                            ps, hT_nb[:, din, j * 128 : (j + 1) * 128], wv_sb[din],
                            start=(din == 0), stop=False,
                        )
                    nc.tensor.matmul(ps, ones_row, bvr, start=False, stop=True)
                    v_sb = workp.tile([128, D], bf16, tag="v_sb")
                    nc.scalar.copy(out=v_sb, in_=ps)
                    nc.gpsimd.dma_start(out=v_dram[kt], in_=v_sb)

            # ---------------- P4: attention ----------------
            o_final = statp.tile([128, RT, D], bf16, tag="o_final")
            for qh in range(2):
                q0 = qh * RH
                out_ps = [pacc.tile([128, 512], fp32, tag="attnacc", name=f"out_ps{qh}_{j}") for j in range(JH)]
                den_ps = pacc.tile([128, 512], fp32, tag="attnacc")
                for kt in range(NT):
                    bt_t = streamp.tile([128, 8, 128], bf16, tag="bt")
                    nc.gpsimd.dma_start(out=bt_t, in_=bt_dram[kt])
                    v_t = streamp.tile([128, D], bf16, tag="vt")
                    nc.gpsimd.dma_start(out=v_t, in_=v_dram[kt])
                    gkx_t = minip.tile([128, 1], fp32, tag="gkx_t")
                    nc.gpsimd.dma_start(out=gkx_t, in_=gkx[kt * 128 : (kt + 1) * 128, :])
                    gky_t = minip.tile([128, 1], fp32, tag="gky_t")
                    nc.gpsimd.dma_start(out=gky_t, in_=gky[kt * 128 : (kt + 1) * 128, :])
                    kidx_t = minip.tile([128, 1], fp32, tag="kidx_t")
                    nc.gpsimd.dma_start(out=kidx_t, in_=kidx[kt * 128 : (kt + 1) * 128, :])

                    lg = pwork.tile([128, RH], fp32, tag="pwork")
                    for dt in range(8):
                        nc.tensor.matmul(
                            lg, bt_t[:, dt, :], qAT[:, dt, q0 : q0 + RH],
                            start=(dt == 0), stop=(dt == 7),
                        )
                    # band mask: max(dx^2, dy^2) <= R^2 and j != i
                    dxa = workp.tile([128, RH], bf16, tag="dxa")
                    nc.vector.tensor_scalar(
                        out=dxa, in0=gqx_bc[:, q0 : q0 + RH], scalar1=gkx_t,
                        scalar2=None, op0=OP.subtract,
                    )
                    nc.vector.tensor_tensor(out=dxa, in0=dxa, in1=dxa, op=OP.mult)
                    dya = workp.tile([128, RH], bf16, tag="dya")
                    nc.vector.tensor_scalar(
                        out=dya, in0=gqy_bc[:, q0 : q0 + RH], scalar1=gky_t,
                        scalar2=None, op0=OP.subtract,
                    )
                    nc.vector.tensor_tensor(out=dya, in0=dya, in1=dya, op=OP.mult)
                    nc.vector.tensor_tensor(out=dxa, in0=dxa, in1=dya, op=OP.max)
                    selfd = workp.tile([128, RH], fp32, tag="selfd")
                    nc.vector.tensor_scalar(
                        out=selfd, in0=qidx_bc[:, q0 : q0 + RH], scalar1=kidx_t,
                        scalar2=None, op0=OP.is_equal,
                    )
                    # chev2' = chev2 + 100*selfeq ; u0 = chev2' <= RADIUS^2
                    nc.vector.scalar_tensor_tensor(
                        out=dxa, in0=selfd, scalar=100.0, in1=dxa, op0=OP.mult, op1=OP.add,
                    )
                    u0 = workp.tile([128, RH], bf16, tag="u0")
                    nc.vector.tensor_scalar(
                        out=u0, in0=dxa, scalar1=float(RADIUS * RADIUS), scalar2=None, op0=OP.is_le,
                    )
                    pm = workp.tile([128, RH], bf16, tag="pm")
                    nc.scalar.activation(out=pm, in_=lg, func=AF.Exp)
                    nc.vector.tensor_tensor(out=pm, in0=pm, in1=u0, op=OP.mult)
                    for j in range(JH):
                        nc.tensor.matmul(
                            out_ps[j], pm[:, j * 128 : (j + 1) * 128], v_t,
                            start=(kt == 0), stop=(kt == NT - 1), skip_group_check=True,
                        )
                        nc.tensor.matmul(
                            den_ps[:, j : j + 1], pm[:, j * 128 : (j + 1) * 128], ones_col,
                            start=(kt == 0), stop=(kt == NT - 1), skip_group_check=True,
                        )
                # epilogue per q-tile
                for j in range(JH):
                    qt = qh * JH + j
                    den = minip.tile([128, 1], fp32, tag="den")
                    nc.vector.tensor_copy(out=den, in_=den_ps[:, j : j + 1])
                    nbr = minip.tile([128, 1], fp32, tag="nbr")
                    nc.vector.tensor_scalar(
                        out=nbr, in0=den, scalar1=0.0, scalar2=None, op0=OP.is_gt,
                    )
                    iso = minip.tile([128, 1], fp32, tag="iso")
                    nc.vector.tensor_scalar(
                        out=iso, in0=nbr, scalar1=-1.0, scalar2=1.0, op0=OP.mult, op1=OP.add,
                    )
                    nc.vector.tensor_tensor(out=den, in0=den, in1=iso, op=OP.add)
                    rden = minip.tile([128, 1], fp32, tag="rden")
                    nc.vector.reciprocal(out=rden, in_=den)
                    on = workp.tile([128, D], bf16, tag="on")
                    nc.vector.tensor_scalar_mul(out=on, in0=out_ps[j], scalar1=rden)
                    nc.vector.scalar_tensor_tensor(
                        out=o_final[:, qt, :], in0=v_own[:, qt, :], scalar=iso, in1=on,
                        op0=OP.mult, op1=OP.add,
                    )

            # ---------------- P5: o@Wo + residual, LN2 ----------------
            oT = statp.tile([128, DT, R], bf16, tag="oT")
            for qt in range(RT):
                for dt in range(DT):
                    transpose_128(o_final[:, qt, dt * 128 : (dt + 1) * 128], oT[:, dt, qt * 128 : (qt + 1) * 128])
            wo_sb = load_w(Wo, DT, D, "wo")
            g2b = None if ln2_identity else g2_bc
            b2b = None if ln2_identity else bb2_bc
            x2 = statp.tile([128, RT, D], bf16, tag="x2")
            h2T = statp.tile([128, DT, R], bf16, tag="h2T")
            for qt in range(RT):
                ps = pwork.tile([128, 512], fp32, tag="pwork")
                for din in range(DT):
                    nc.tensor.matmul(
                        ps, oT[:, din, qt * 128 : (qt + 1) * 128], wo_sb[din],
                        start=(din == 0), stop=False,
                    )
                nc.tensor.matmul(ps, ones_row, bor, start=False, stop=True)
                xq_t = workp.tile([128, D], fp32, tag="xq_t")
                nc.gpsimd.dma_start(out=xq_t, in_=xq[qt * 128 : (qt + 1) * 128, :])
                nc.vector.tensor_tensor(out=x2[:, qt, :], in0=ps, in1=xq_t, op=OP.add)
                h2_t, _ = layer_norm(x2[:, qt, :], 128, ln2_identity, g2b, b2b)
                for dt in range(DT):
                    transpose_128(h2_t[:, dt * 128 : (dt + 1) * 128], h2T[:, dt, qt * 128 : (qt + 1) * 128])

            # ---------------- P6: FFN ----------------
            w1_sb = load_w(W1, DT, 4 * D, "w1")
            aT = statp.tile([128, FT, R], bf16, tag="aT")
            for ft in range(FT):
                for c0, cw in ((0, 512), (512, R - 512)) if R > 512 else ((0, R),):
                    ps = pwork.tile([128, 512], fp32, tag="pwork")
                    for din in range(DT):
                        nc.tensor.matmul(
                            ps[:, :cw], w1_sb[din][:, ft * 128 : (ft + 1) * 128],
                            h2T[:, din, c0 : c0 + cw],
                            start=(din == 0), stop=(din == DT - 1),
                        )
                    if gelu_mode == "hw":
                        nc.scalar.activation(
                            out=aT[:, ft, c0 : c0 + cw], in_=ps[:, :cw], func=AF.Gelu,
                            bias=b1s[:, ft : ft + 1],
                        )
                    else:
                        # sim-testable exact-enough gelu: 0.5x(1+tanh(.79788(x+.044715x^3)))
                        xg = workp.tile([128, 512], fp32, tag="xg")
                        nc.vector.tensor_scalar(
                            out=xg[:, :cw], in0=ps[:, :cw], scalar1=b1s[:, ft : ft + 1],
                            scalar2=None, op0=OP.add,
                        )
                        u2 = workp.tile([128, 512], fp32, tag="u2")
                        nc.scalar.activation(out=u2[:, :cw], in_=xg[:, :cw], func=AF.Square)
                        nc.vector.tensor_scalar(
                            out=u2[:, :cw], in0=u2[:, :cw], scalar1=0.044715,
                            scalar2=1.0, op0=OP.mult, op1=OP.add,
                        )
                        nc.vector.tensor_tensor(out=u2[:, :cw], in0=u2[:, :cw], in1=xg[:, :cw], op=OP.mult)
                        nc.scalar.activation(out=u2[:, :cw], in_=u2[:, :cw], func=AF.Tanh, scale=0.7978845608028654)
                        nc.vector.tensor_scalar(
                            out=u2[:, :cw], in0=u2[:, :cw], scalar1=1.0,
                            scalar2=0.5, op0=OP.add, op1=OP.mult,
                        )
                        nc.vector.tensor_tensor(out=aT[:, ft, c0 : c0 + cw], in0=u2[:, :cw], in1=xg[:, :cw], op=OP.mult)
            w2_sb = []
            for i in range(DT):
                t = wpool.tile([128, 4, D], bf16, tag="w", name=f"w2_{i}")
                nc.gpsimd.dma_start(
                    out=t,
                    in_=W2[i * 512 : (i + 1) * 512, :].rearrange("(a p) b -> p a b", p=128),
                )
                w2_sb.append(t)
            for qt in range(RT):
                ps = pacc.tile([128, 512], fp32, tag="attnacc")
                for ft in range(FT):
                    nc.tensor.matmul(
                        ps, aT[:, ft, qt * 128 : (qt + 1) * 128], w2_sb[ft // 4][:, ft % 4, :],
                        start=(ft == 0), stop=False,
                    )
                nc.tensor.matmul(ps, ones_row, b2r, start=False, stop=True)
                out_t = workp.tile([128, D], fp32, tag="out_t")
                nc.vector.tensor_tensor(out=out_t, in0=ps, in1=x2[:, qt, :], op=OP.add)
                nc.gpsimd.dma_start(out=out[qt * 128 : (qt + 1) * 128, :], in_=out_t)

    nc.compile()
    return nc


def _get_program(ln1_identity, ln2_identity, gelu_mode="hw"):
    key = (ln1_identity, ln2_identity, gelu_mode)
    if key not in _PROG_CACHE:
        _PROG_CACHE[key] = _build_program(ln1_identity, ln2_identity, gelu_mode)
    return _PROG_CACHE[key]


def _make_in_maps(x, grid, Wq, bq, Wk, bk, Wv, bv, Wo, bo,
                  ln1_g, ln1_b, ln2_g, ln2_b, W1, b1, W2, b2,
                  ln1_identity, ln2_identity):
    import ml_dtypes

    bf = ml_dtypes.bfloat16
    f32 = np.float32
    x = np.asarray(x, f32)
    g = np.asarray(grid).astype(f32)

    shared = dict(
        x_bf=x.astype(bf),
        gkx=np.ascontiguousarray(g[:, 0:1]),
        gky=np.ascontiguousarray(g[:, 1:2]),
        kidx=np.arange(N, dtype=f32).reshape(N, 1),
        Wq=np.asarray(Wq, f32).astype(bf), Wk=np.asarray(Wk, f32).astype(bf),
        Wv=np.asarray(Wv, f32).astype(bf), Wo=np.asarray(Wo, f32).astype(bf),
        W1=np.asarray(W1, f32).astype(bf), W2=np.asarray(W2, f32).astype(bf),
        bq_c=np.asarray(bq, f32).reshape(D, 1),
        bk_c=np.asarray(bk, f32).reshape(D, 1),
        b1_c=np.asarray(b1, f32).reshape(4 * D, 1),
        bv_r=np.asarray(bv, f32).reshape(1, D).astype(bf),
        bo_r=np.asarray(bo, f32).reshape(1, D).astype(bf),
        b2_r=np.asarray(b2, f32).reshape(1, D).astype(bf),
    )
    if not ln1_identity:
        shared["g1_r"] = np.asarray(ln1_g, f32).reshape(1, D)
        shared["bb1_r"] = np.asarray(ln1_b, f32).reshape(1, D)
    if not ln2_identity:
        shared["g2_r"] = np.asarray(ln2_g, f32).reshape(1, D)
        shared["bb2_r"] = np.asarray(ln2_b, f32).reshape(1, D)

    in_maps = []
    for s in range(8):
        r0 = s * R
        m = dict(shared)
        m["xq"] = np.ascontiguousarray(x[r0 : r0 + R])
        m["gqx"] = np.ascontiguousarray(g[r0 : r0 + R, 0]).reshape(1, R).astype(bf)
        m["gqy"] = np.ascontiguousarray(g[r0 : r0 + R, 1]).reshape(1, R).astype(bf)
        m["qidx"] = np.arange(r0, r0 + R, dtype=f32).reshape(1, R)
        in_maps.append(m)
    return in_maps


def kernel(x, grid, Wq, bq, Wk, bk, Wv, bv, Wo, bo,
           ln1_g, ln1_b, ln2_g, ln2_b, W1, b1, W2, b2,
           _trace=False):
    from concourse.bass_utils import run_bass_kernel_spmd

    ln1_identity = bool(np.all(np.asarray(ln1_g) == 1.0) and np.all(np.asarray(ln1_b) == 0.0))
    ln2_identity = bool(np.all(np.asarray(ln2_g) == 1.0) and np.all(np.asarray(ln2_b) == 0.0))
    nc = _get_program(ln1_identity, ln2_identity)
    in_maps = _make_in_maps(x, grid, Wq, bq, Wk, bk, Wv, bv, Wo, bo,
                            ln1_g, ln1_b, ln2_g, ln2_b, W1, b1, W2, b2,
                            ln1_identity, ln2_identity)
    res = run_bass_kernel_spmd(nc, in_maps, core_ids=list(range(8)), trace=_trace)
    outp = np.concatenate([res.results[s]["out"] for s in range(8)], axis=0)
    kernel.last_result = res
    return outp.astype(np.float32)
